# revision 16
# baseline (speedup 1.0000x reference)
"""Trainium2 Bass kernel for nn_LinkPredictor.

Reference computation (B=4, N=256, T=16, F=128, H=256):
    h = mean_T(nodefeat)                      # [B,N,F]
    a = h @ W1[:, :F].T                       # [B,N,H]
    c = h @ W1[:, F:].T                       # [B,N,H]
    logits[b,i,j] = W2[0] . relu(a[b,i] + c[b,j] + b1) + b2   # [B,N,N]

Sharding: 8 cores; core k handles batch b=k//2, i-half k%2 (128 i-rows x
256 j-cols of one batch's NxN grid).

Per-core plan (v5):
  - nf and the mean-selection matrix in fp8-e4m3 (error budget checked in
    sim; DMA bytes halve vs bf16). 5 nf chunks + weights balanced across
    3 trigger queues (sync HW, gpsimd SW, scalar HW) at ~256KB each.
  - hT via per-octet matmuls (fp8 stationary x fp8 moving) pipelined
    behind the DMA chunks, drained bf16 in j-128 halves; cT matmuls and
    ScalarE copies chunked so the first act op trails the last nf byte
    by ~1us.
  - aTb4 = (aT+b1) x4-replicated fp32 for the VE 16B-aligned scalar
    fast path.
  - Pairwise: act buffer [128, 1024] per pair-step = [ht0 i_a | ht0 i_b
    | ht1 i_a | ht1 i_b], filled by 4 ops split between VectorE
    (dual-op tensor_scalar ~203ns) and ScalarE (activation Relu+bias
    ~412ns) via greedy finish-time balancing.
  - Reduction: one matmul per (pair, ht): stationary [128,n_g] diag with
    w2_ht in column r', PSUM region [n_g, 512] per group; group sizes
    16/16/16/12/4 so drains overlap compute and the final
    drain+DMA tail is small.
  - b2 and final assembly applied on host.
"""

import os
import sys

import numpy as np

_B, _N, _T, _F, _H = 4, 256, 16, 128, 256
_NCORES = 8

_VE_NS = 202.7  # measured per [128,256] dual tensor_scalar
_SE_NS = 411.5  # measured per [128,256] activation relu+bias
_SE_DRAIN_NS = 720.0
_VE_DRAIN_NS = 658.0
_GP_NS = 3852.0  # measured per [128,256] gpsimd dual tensor_scalar

_GROUPS = [16, 16, 16, 12, 4]  # pairs per PSUM region

_CACHE = {}


def _ensure_paths():
    for p in (
        "/root/.axon_site",
        "/root/.axon_site/_ro/trn_rl_repo",
        "/root/.axon_site/_ro/pypackages",
        "/opt/trn_rl_repo",
    ):
        if os.path.isdir(p) and p not in sys.path:
            sys.path.append(p)


def build_nc():
    """Build the per-core Bass program (same program for all 8 cores)."""
    _ensure_paths()
    import concourse.mybir as mybir
    import concourse.tile as tile
    from concourse import bacc

    f32 = mybir.dt.float32
    bf16 = mybir.dt.bfloat16
    fp8 = mybir.dt.float8e4
    Alu = mybir.AluOpType
    Act = mybir.ActivationFunctionType

    nc = bacc.Bacc("TRN2", target_bir_lowering=False, debug=False)

    nf = nc.declare_dram_parameter("nf", [128, 32, 128], fp8, isOutput=False)
    smat = nc.declare_dram_parameter("smat", [128, 8], fp8, isOutput=False)
    # wpack[:, t, 0:128]=w1c_t^T, [:, t, 128:256]=w1a_t^T, [:, t, 256:512]=w2 diag
    wpack = nc.declare_dram_parameter("wpack", [128, 2, 512], bf16, isOutput=False)
    b1t = nc.declare_dram_parameter("b1t", [128, 2], f32, isOutput=False)
    outd = nc.declare_dram_parameter("out", [64, 512], f32, isOutput=True)

    with tile.TileContext(nc) as tc:
        with (
            tc.tile_pool(name="const", bufs=1) as constp,
            tc.tile_pool(name="data", bufs=1) as datap,
            tc.tile_pool(name="act", bufs=20) as actp,
            tc.tile_pool(name="dr", bufs=2) as drp,
            tc.tile_pool(name="ph", bufs=1, space="PSUM") as php,
            tc.tile_pool(name="pc", bufs=2, space="PSUM") as pcp,
            tc.tile_pool(name="pl", bufs=2, space="PSUM") as plp,
        ):
            smat_sb = constp.tile([128, 8], fp8, tag="smat")
            nc.sync.dma_start(out=smat_sb[:], in_=smat[:])

            nf_sb = constp.tile([128, 32, 128], fp8, tag="nf")
            wpack_sb = constp.tile([128, 2, 512], bf16, tag="wpack")
            b1t_sb = constp.tile([128, 2], f32, tag="b1t")
            # Queue plan (per-queue ~1.6us latency + ~85GB/s): b1t + the w1
            # half of wpack lead the scalar queue (needed first); nf split
            # ~byte-balanced across all three queues; w2 diag half arrives
            # before the first pairwise matmul.
            nc.scalar.dma_start(out=b1t_sb[:], in_=b1t[:])
            nc.scalar.dma_start(out=wpack_sb[:, :, 0:256], in_=wpack[:, :, 0:256])
            nc.sync.dma_start(out=nf_sb[:, 0:6, :], in_=nf[:, 0:6, :])
            nc.gpsimd.dma_start(out=nf_sb[:, 11:17, :], in_=nf[:, 11:17, :])
            nc.sync.dma_start(out=nf_sb[:, 6:11, :], in_=nf[:, 6:11, :])
            nc.gpsimd.dma_start(out=nf_sb[:, 17:22, :], in_=nf[:, 17:22, :])
            nc.scalar.dma_start(out=nf_sb[:, 22:32, :], in_=nf[:, 22:32, :])
            nc.scalar.dma_start(out=wpack_sb[:, :, 256:512], in_=wpack[:, :, 256:512])

            # split PSUM tiles so j-half consumers wait only their own half;
            # octet order follows expected DMA chunk arrival
            phA = php.tile([128, 128], f32, tag="phA")
            phB = php.tile([128, 128], f32, tag="phB")
            oct_order = (
                list(range(0, 6)) + list(range(11, 16)) + list(range(6, 11))
                + [16] + list(range(17, 22)) + list(range(22, 32))
            )
            for o in oct_order:
                ph = phA if o < 16 else phB
                nc.tensor.matmul(
                    ph[:, 8 * (o % 16) : 8 * (o % 16) + 8],
                    lhsT=nf_sb[:, o, :],
                    rhs=smat_sb[:],
                    start=True,
                    stop=True,
                )

            hTa = datap.tile([128, 128], bf16, tag="hTa")
            hTb = datap.tile([128, 128], bf16, tag="hTb")
            cT = [datap.tile([128, 256], bf16, tag=f"cT{t}", name=f"cT{t}") for t in range(2)]
            aTb4 = [datap.tile([128, 128, 4], f32, tag=f"aTb4{t}", name=f"aTb4{t}") for t in range(2)]
            # separate [128,128] PSUM tiles per (t, j-half): no false WAR deps
            pcs = [
                [pcp.tile([128, 128], f32, tag="pc", name=f"pc{t}h{h}") for h in range(2)]
                for t in range(2)
            ]

            # first j-half: hT drain, cT chunk mms, aT chain, cT copies
            nc.vector.tensor_copy(hTa[:], phA[:])
            for t in range(2):
                nc.tensor.matmul(
                    pcs[t][0][:], lhsT=wpack_sb[:, t, 0:128],
                    rhs=hTa[:], start=True, stop=True,
                )
            pa = [pcp.tile([128, 128], f32, tag="pa", name=f"pa{t}") for t in range(2)]
            for t in range(2):
                nc.tensor.matmul(
                    pa[t][:], lhsT=wpack_sb[:, t, 128:256],
                    rhs=hTa[:], start=True, stop=True,
                )
            # second j-half feeds through before the aTb4 build so the
            # critical path (cT[0] complete) clears ScalarE early
            nc.vector.tensor_copy(hTb[:], phB[:])
            for t in range(2):
                nc.tensor.matmul(
                    pcs[t][1][:], lhsT=wpack_sb[:, t, 0:128],
                    rhs=hTb[:], start=True, stop=True,
                )
            nc.scalar.copy(cT[0][:, 0:128], pcs[0][0][:])
            nc.scalar.copy(cT[0][:, 128:256], pcs[0][1][:])
            nc.vector.tensor_scalar(
                aTb4[0][:, :, :],
                pa[0][:].broadcast_to([128, 128, 4]),
                b1t_sb[:, 0:1],
                None,
                Alu.add,
            )
            nc.scalar.copy(cT[1][:, 0:128], pcs[1][0][:])
            nc.scalar.copy(cT[1][:, 128:256], pcs[1][1][:])
            nc.vector.tensor_scalar(
                aTb4[1][:, :, :],
                pa[1][:].broadcast_to([128, 128, 4]),
                b1t_sb[:, 1:2],
                None,
                Alu.add,
            )

            # Pairwise main loop over groups of pairs.  Each matmul's moving
            # operand is one [128,512] tile written by exactly one engine
            # (two ops) — no cross-engine intra-tile deps.  GpSimd takes an
            # occasional tile pair as a slow third engine.
            tV = 700.0  # aTb4[0] ahead of first act
            tS = 400.0  # cT copies
            tG = 0.0
            p0 = 0
            for g, ng in enumerate(_GROUPS):
                last_group = g == len(_GROUPS) - 1
                pl = plp.tile([ng, 512], f32, tag="pl", name=f"pl{g}")
                for rp in range(ng):
                    for t in range(2):
                        buf = actp.tile([128, 512], bf16, tag="act")
                        cV = tV + 2 * _VE_NS
                        cS = tS + 2 * _SE_NS
                        cG = tG + 2 * _GP_NS
                        best = min(cV, cS, cG)
                        if best == cV:
                            eng, tV = nc.vector, cV
                        elif best == cS:
                            eng, tS = nc.scalar, cS
                        else:
                            eng, tG = nc.gpsimd, cG
                        for s in range(2):
                            i = 2 * (p0 + rp) + s
                            a_col = aTb4[t][:, i, 0:1]
                            dst = buf[:, 256 * s : 256 * s + 256]
                            if eng is nc.scalar:
                                nc.scalar.activation(dst, cT[t][:], Act.Relu, bias=a_col)
                            else:
                                eng.tensor_scalar(
                                    dst, cT[t][:], a_col, 0.0, Alu.add, Alu.max
                                )
                        nc.tensor.matmul(
                            pl[:, :],
                            lhsT=wpack_sb[:, t, 256 + 16 * rp : 256 + 16 * rp + ng],
                            rhs=buf[:],
                            start=(rp == 0 and t == 0),
                            stop=(rp == ng - 1 and t == 1),
                        )
                osb = drp.tile([ng, 512], f32, tag="osb", name=f"osb{g}")
                if last_group or tV + _VE_DRAIN_NS <= tS + _SE_DRAIN_NS:
                    nc.vector.tensor_copy(osb[:], pl[:])
                    tV += _VE_DRAIN_NS
                else:
                    nc.scalar.copy(osb[:], pl[:])
                    tS += _SE_DRAIN_NS
                nc.sync.dma_start(out=outd[p0 : p0 + ng], in_=osb[:])
                p0 += ng

    nc.compile()
    return nc


def make_in_maps(nodefeat, W1, b1, W2, b2):
    """Host-side sharding/layout prep (layout + dtype only)."""
    import ml_dtypes

    bf16 = ml_dtypes.bfloat16
    fp8 = ml_dtypes.float8_e4m3fn
    nodefeat = np.asarray(nodefeat, dtype=np.float32)
    W1 = np.asarray(W1, dtype=np.float32)
    b1 = np.asarray(b1, dtype=np.float32)
    W2 = np.asarray(W2, dtype=np.float32)

    smat = (np.repeat(np.eye(8, dtype=np.float32), 16, axis=0) / 16.0).astype(fp8)

    W1a, W1c = W1[:, :_F], W1[:, _F:]
    w1at = np.stack([W1a[:128].T, W1a[128:].T], axis=1)  # [128 f, 2, 128 h]
    w1ct = np.stack([W1c[:128].T, W1c[128:].T], axis=1)
    b1t = np.ascontiguousarray(b1.reshape(2, 128).T)

    w2r = W2[0].reshape(2, 128)  # [ht, p]
    w2b = np.zeros((128, 2, 16, 16), dtype=np.float32)
    idx = np.arange(16)
    w2b[:, :, idx, idx] = w2r.T[:, :, None]

    wpack = np.concatenate(
        [w1ct, w1at, w2b.reshape(128, 2, 256)], axis=2
    ).astype(bf16)  # [128, 2, 512]

    # fp8 with error feedback along T: each slice is individually fp8-close
    # to its true value, and the T-sum the device computes stays accurate.
    nfq = np.empty_like(nodefeat)
    carry = np.zeros(nodefeat[:, :, 0, :].shape, dtype=np.float32)
    for t in range(_T):
        x = nodefeat[:, :, t, :] + carry
        qx = x.astype(fp8).astype(np.float32)
        carry = x - qx
        nfq[:, :, t, :] = qx

    in_maps = []
    for k in range(_NCORES):
        b, ih = divmod(k, 2)
        nf_b = nfq[b]  # [256, 16, 128]
        if ih:
            nf_b = np.concatenate([nf_b[128:], nf_b[:128]], axis=0)
        # [256,16,128] -> [32 oct, (j8,t16)=128, 128 f] -> [128, 32, 128]
        nf_dev = np.ascontiguousarray(
            nf_b.reshape(32, 128, 128).transpose(1, 0, 2).astype(fp8)
        )
        in_maps.append(
            {
                "nf": nf_dev,
                "smat": smat,
                "wpack": wpack,
                "b1t": b1t,
            }
        )
    return in_maps


def core_output_to_ij(arr, b2_val):
    """Device output [64, 512] -> core-local logits [128 i, 256 j]."""
    return arr.reshape(128, 256).astype(np.float32) + b2_val


def assemble_output(results, b2):
    b2_val = float(np.asarray(b2).reshape(-1)[0])
    out = np.empty((_B, _N, _N), dtype=np.float32)
    for k in range(_NCORES):
        b, ih = divmod(k, 2)
        r = core_output_to_ij(results[k]["out"], b2_val)  # [i, j] core-local j order
        if ih:
            r = np.concatenate([r[:, 128:], r[:, :128]], axis=1)
        out[b, ih * 128 : (ih + 1) * 128, :] = r
    return out


def _get_nc():
    if "nc" not in _CACHE:
        _CACHE["nc"] = build_nc()
    return _CACHE["nc"]


def kernel(nodefeat, W1, b1, W2, b2):
    _ensure_paths()
    from concourse.bass_utils import run_bass_kernel_spmd

    nc = _get_nc()
    in_maps = make_in_maps(nodefeat, W1, b1, W2, b2)
    res = run_bass_kernel_spmd(nc, in_maps, list(range(_NCORES)))
    return assemble_output(res.results, b2)


# revision 17
# speedup vs baseline: 1.4685x; 1.4685x over previous
"""Trainium2 Bass kernel for nn_LinkPredictor.

Reference computation (B=4, N=256, T=16, F=128, H=256):
    h = mean_T(nodefeat)                      # [B,N,F]
    a = h @ W1[:, :F].T                       # [B,N,H]
    c = h @ W1[:, F:].T                       # [B,N,H]
    logits[b,i,j] = W2[0] . relu(a[b,i] + c[b,j] + b1) + b2   # [B,N,N]

Sharding: 8 cores; core k handles batch b=k//2, i-half k%2 (128 i-rows x
256 j-cols of one batch's NxN grid).

Per-core plan (v5):
  - nf and the mean-selection matrix in fp8-e4m3 (error budget checked in
    sim; DMA bytes halve vs bf16). 5 nf chunks + weights balanced across
    3 trigger queues (sync HW, gpsimd SW, scalar HW) at ~256KB each.
  - hT via per-octet matmuls (fp8 stationary x fp8 moving) pipelined
    behind the DMA chunks, drained bf16 in j-128 halves; cT matmuls and
    ScalarE copies chunked so the first act op trails the last nf byte
    by ~1us.
  - aTb4 = (aT+b1) x4-replicated fp32 for the VE 16B-aligned scalar
    fast path.
  - Pairwise: act buffer [128, 1024] per pair-step = [ht0 i_a | ht0 i_b
    | ht1 i_a | ht1 i_b], filled by 4 ops split between VectorE
    (dual-op tensor_scalar ~203ns) and ScalarE (activation Relu+bias
    ~412ns) via greedy finish-time balancing.
  - Reduction: one matmul per (pair, ht): stationary [128,n_g] diag with
    w2_ht in column r', PSUM region [n_g, 512] per group; group sizes
    16/16/16/12/4 so drains overlap compute and the final
    drain+DMA tail is small.
  - b2 and final assembly applied on host.
"""

import os
import sys

import numpy as np

_B, _N, _T, _F, _H = 4, 256, 16, 128, 256
_NCORES = 8

_VE_NS = 202.7  # measured per [128,256] dual tensor_scalar
_SE_NS = 411.5  # measured per [128,256] activation relu+bias
_SE_DRAIN_NS = 720.0
_VE_DRAIN_NS = 658.0
_GP_NS = 1e9  # gpsimd act tiles disabled: in-context cost blew up

_GROUPS = [16, 16, 16, 12, 4]  # pairs per PSUM region

_CACHE = {}


def _ensure_paths():
    for p in (
        "/root/.axon_site",
        "/root/.axon_site/_ro/trn_rl_repo",
        "/root/.axon_site/_ro/pypackages",
        "/opt/trn_rl_repo",
    ):
        if os.path.isdir(p) and p not in sys.path:
            sys.path.append(p)


def build_nc():
    """Build the per-core Bass program (same program for all 8 cores)."""
    _ensure_paths()
    import concourse.mybir as mybir
    import concourse.tile as tile
    from concourse import bacc

    f32 = mybir.dt.float32
    bf16 = mybir.dt.bfloat16
    fp8 = mybir.dt.float8e4
    Alu = mybir.AluOpType
    Act = mybir.ActivationFunctionType

    nc = bacc.Bacc("TRN2", target_bir_lowering=False, debug=False)

    nf = nc.declare_dram_parameter("nf", [128, 32, 128], fp8, isOutput=False)
    smat = nc.declare_dram_parameter("smat", [128, 8], fp8, isOutput=False)
    # wpack[:, t, 0:128]=w1c_t^T, [:, t, 128:256]=w1a_t^T, [:, t, 256:512]=w2 diag
    wpack = nc.declare_dram_parameter("wpack", [128, 2, 512], bf16, isOutput=False)
    b1t = nc.declare_dram_parameter("b1t", [128, 2], f32, isOutput=False)
    outd = nc.declare_dram_parameter("out", [64, 512], f32, isOutput=True)

    with tile.TileContext(nc) as tc:
        with (
            tc.tile_pool(name="const", bufs=1) as constp,
            tc.tile_pool(name="data", bufs=1) as datap,
            tc.tile_pool(name="act", bufs=20) as actp,
            tc.tile_pool(name="dr", bufs=2) as drp,
            tc.tile_pool(name="ph", bufs=1, space="PSUM") as php,
            tc.tile_pool(name="pc", bufs=2, space="PSUM") as pcp,
            tc.tile_pool(name="pl", bufs=2, space="PSUM") as plp,
        ):
            smat_sb = constp.tile([128, 8], fp8, tag="smat")
            nc.sync.dma_start(out=smat_sb[:], in_=smat[:])

            nf_sb = constp.tile([128, 32, 128], fp8, tag="nf")
            wpack_sb = constp.tile([128, 2, 512], bf16, tag="wpack")
            b1t_sb = constp.tile([128, 2], f32, tag="b1t")
            # Queue plan (per-queue ~1.6us latency + ~85GB/s): b1t + the w1
            # half of wpack lead the scalar queue (needed first); nf split
            # ~byte-balanced across all three queues; w2 diag half arrives
            # before the first pairwise matmul.
            nc.scalar.dma_start(out=b1t_sb[:], in_=b1t[:])
            nc.scalar.dma_start(out=wpack_sb[:, :, 0:256], in_=wpack[:, :, 0:256])
            nc.sync.dma_start(out=nf_sb[:, 0:6, :], in_=nf[:, 0:6, :])
            nc.gpsimd.dma_start(out=nf_sb[:, 11:17, :], in_=nf[:, 11:17, :])
            nc.sync.dma_start(out=nf_sb[:, 6:11, :], in_=nf[:, 6:11, :])
            nc.gpsimd.dma_start(out=nf_sb[:, 17:22, :], in_=nf[:, 17:22, :])
            nc.scalar.dma_start(out=nf_sb[:, 22:32, :], in_=nf[:, 22:32, :])
            nc.scalar.dma_start(out=wpack_sb[:, :, 256:512], in_=wpack[:, :, 256:512])

            # split PSUM tiles so j-half consumers wait only their own half;
            # octet order follows expected DMA chunk arrival
            phA = php.tile([128, 128], f32, tag="phA")
            phB = php.tile([128, 128], f32, tag="phB")
            oct_order = (
                list(range(0, 6)) + list(range(11, 16)) + list(range(6, 11))
                + [16] + list(range(17, 22)) + list(range(22, 32))
            )
            for o in oct_order:
                ph = phA if o < 16 else phB
                nc.tensor.matmul(
                    ph[:, 8 * (o % 16) : 8 * (o % 16) + 8],
                    lhsT=nf_sb[:, o, :],
                    rhs=smat_sb[:],
                    start=True,
                    stop=True,
                )

            hTa = datap.tile([128, 128], bf16, tag="hTa")
            hTb = datap.tile([128, 128], bf16, tag="hTb")
            cT = [datap.tile([128, 256], bf16, tag=f"cT{t}", name=f"cT{t}") for t in range(2)]
            aTb4 = [datap.tile([128, 128, 4], f32, tag=f"aTb4{t}", name=f"aTb4{t}") for t in range(2)]
            # separate [128,128] PSUM tiles per (t, j-half): no false WAR deps
            pcs = [
                [pcp.tile([128, 128], f32, tag="pc", name=f"pc{t}h{h}") for h in range(2)]
                for t in range(2)
            ]

            # first j-half: hT drain, cT chunk mms, aT chain, cT copies
            nc.vector.tensor_copy(hTa[:], phA[:])
            for t in range(2):
                nc.tensor.matmul(
                    pcs[t][0][:], lhsT=wpack_sb[:, t, 0:128],
                    rhs=hTa[:], start=True, stop=True,
                )
            pa = [pcp.tile([128, 128], f32, tag="pa", name=f"pa{t}") for t in range(2)]
            for t in range(2):
                nc.tensor.matmul(
                    pa[t][:], lhsT=wpack_sb[:, t, 128:256],
                    rhs=hTa[:], start=True, stop=True,
                )
            # second j-half feeds through before the aTb4 build so the
            # critical path (cT[0] complete) clears ScalarE early
            nc.vector.tensor_copy(hTb[:], phB[:])
            for t in range(2):
                nc.tensor.matmul(
                    pcs[t][1][:], lhsT=wpack_sb[:, t, 0:128],
                    rhs=hTb[:], start=True, stop=True,
                )
            nc.scalar.copy(cT[0][:, 0:128], pcs[0][0][:])
            nc.scalar.copy(cT[0][:, 128:256], pcs[0][1][:])
            nc.vector.tensor_scalar(
                aTb4[0][:, :, :],
                pa[0][:].broadcast_to([128, 128, 4]),
                b1t_sb[:, 0:1],
                None,
                Alu.add,
            )
            nc.scalar.copy(cT[1][:, 0:128], pcs[1][0][:])
            nc.scalar.copy(cT[1][:, 128:256], pcs[1][1][:])
            nc.vector.tensor_scalar(
                aTb4[1][:, :, :],
                pa[1][:].broadcast_to([128, 128, 4]),
                b1t_sb[:, 1:2],
                None,
                Alu.add,
            )

            # Pairwise main loop over groups of pairs.  Each matmul's moving
            # operand is one [128,512] tile written by exactly one engine
            # (two ops) — no cross-engine intra-tile deps.  GpSimd takes an
            # occasional tile pair as a slow third engine.
            tV = 700.0  # aTb4[0] ahead of first act
            tS = 400.0  # cT copies
            tG = 0.0
            p0 = 0
            for g, ng in enumerate(_GROUPS):
                last_group = g == len(_GROUPS) - 1
                pl = plp.tile([ng, 512], f32, tag="pl", name=f"pl{g}")
                for rp in range(ng):
                    for t in range(2):
                        buf = actp.tile([128, 512], bf16, tag="act")
                        cV = tV + 2 * _VE_NS
                        cS = tS + 2 * _SE_NS
                        cG = tG + 2 * _GP_NS
                        best = min(cV, cS, cG)
                        if best == cV:
                            eng, tV = nc.vector, cV
                        elif best == cS:
                            eng, tS = nc.scalar, cS
                        else:
                            eng, tG = nc.gpsimd, cG
                        for s in range(2):
                            i = 2 * (p0 + rp) + s
                            a_col = aTb4[t][:, i, 0:1]
                            dst = buf[:, 256 * s : 256 * s + 256]
                            if eng is nc.scalar:
                                nc.scalar.activation(dst, cT[t][:], Act.Relu, bias=a_col)
                            else:
                                eng.tensor_scalar(
                                    dst, cT[t][:], a_col, 0.0, Alu.add, Alu.max
                                )
                        nc.tensor.matmul(
                            pl[:, :],
                            lhsT=wpack_sb[:, t, 256 + 16 * rp : 256 + 16 * rp + ng],
                            rhs=buf[:],
                            start=(rp == 0 and t == 0),
                            stop=(rp == ng - 1 and t == 1),
                        )
                osb = drp.tile([ng, 512], f32, tag="osb", name=f"osb{g}")
                if last_group or tV + _VE_DRAIN_NS <= tS + _SE_DRAIN_NS:
                    nc.vector.tensor_copy(osb[:], pl[:])
                    tV += _VE_DRAIN_NS
                else:
                    nc.scalar.copy(osb[:], pl[:])
                    tS += _SE_DRAIN_NS
                nc.sync.dma_start(out=outd[p0 : p0 + ng], in_=osb[:])
                p0 += ng

    nc.compile()
    return nc


def make_in_maps(nodefeat, W1, b1, W2, b2):
    """Host-side sharding/layout prep (layout + dtype only)."""
    import ml_dtypes

    bf16 = ml_dtypes.bfloat16
    fp8 = ml_dtypes.float8_e4m3fn
    nodefeat = np.asarray(nodefeat, dtype=np.float32)
    W1 = np.asarray(W1, dtype=np.float32)
    b1 = np.asarray(b1, dtype=np.float32)
    W2 = np.asarray(W2, dtype=np.float32)

    smat = (np.repeat(np.eye(8, dtype=np.float32), 16, axis=0) / 16.0).astype(fp8)

    W1a, W1c = W1[:, :_F], W1[:, _F:]
    w1at = np.stack([W1a[:128].T, W1a[128:].T], axis=1)  # [128 f, 2, 128 h]
    w1ct = np.stack([W1c[:128].T, W1c[128:].T], axis=1)
    b1t = np.ascontiguousarray(b1.reshape(2, 128).T)

    w2r = W2[0].reshape(2, 128)  # [ht, p]
    w2b = np.zeros((128, 2, 16, 16), dtype=np.float32)
    idx = np.arange(16)
    w2b[:, :, idx, idx] = w2r.T[:, :, None]

    wpack = np.concatenate(
        [w1ct, w1at, w2b.reshape(128, 2, 256)], axis=2
    ).astype(bf16)  # [128, 2, 512]

    # fp8 with error feedback along T: each slice is individually fp8-close
    # to its true value, and the T-sum the device computes stays accurate.
    nfq = np.empty_like(nodefeat)
    carry = np.zeros(nodefeat[:, :, 0, :].shape, dtype=np.float32)
    for t in range(_T):
        x = nodefeat[:, :, t, :] + carry
        qx = x.astype(fp8).astype(np.float32)
        carry = x - qx
        nfq[:, :, t, :] = qx

    in_maps = []
    for k in range(_NCORES):
        b, ih = divmod(k, 2)
        nf_b = nfq[b]  # [256, 16, 128]
        if ih:
            nf_b = np.concatenate([nf_b[128:], nf_b[:128]], axis=0)
        # [256,16,128] -> [32 oct, (j8,t16)=128, 128 f] -> [128, 32, 128]
        nf_dev = np.ascontiguousarray(
            nf_b.reshape(32, 128, 128).transpose(1, 0, 2).astype(fp8)
        )
        in_maps.append(
            {
                "nf": nf_dev,
                "smat": smat,
                "wpack": wpack,
                "b1t": b1t,
            }
        )
    return in_maps


def core_output_to_ij(arr, b2_val):
    """Device output [64, 512] -> core-local logits [128 i, 256 j]."""
    return arr.reshape(128, 256).astype(np.float32) + b2_val


def assemble_output(results, b2):
    b2_val = float(np.asarray(b2).reshape(-1)[0])
    out = np.empty((_B, _N, _N), dtype=np.float32)
    for k in range(_NCORES):
        b, ih = divmod(k, 2)
        r = core_output_to_ij(results[k]["out"], b2_val)  # [i, j] core-local j order
        if ih:
            r = np.concatenate([r[:, 128:], r[:, :128]], axis=1)
        out[b, ih * 128 : (ih + 1) * 128, :] = r
    return out


def _get_nc():
    if "nc" not in _CACHE:
        _CACHE["nc"] = build_nc()
    return _CACHE["nc"]


def kernel(nodefeat, W1, b1, W2, b2):
    _ensure_paths()
    from concourse.bass_utils import run_bass_kernel_spmd

    nc = _get_nc()
    in_maps = make_in_maps(nodefeat, W1, b1, W2, b2)
    res = run_bass_kernel_spmd(nc, in_maps, list(range(_NCORES)))
    return assemble_output(res.results, b2)


# revision 20
# speedup vs baseline: 1.4740x; 1.0037x over previous
"""Trainium2 Bass kernel for nn_LinkPredictor.

Reference computation (B=4, N=256, T=16, F=128, H=256):
    h = mean_T(nodefeat)                      # [B,N,F]
    a = h @ W1[:, :F].T                       # [B,N,H]
    c = h @ W1[:, F:].T                       # [B,N,H]
    logits[b,i,j] = W2[0] . relu(a[b,i] + c[b,j] + b1) + b2   # [B,N,N]

Sharding: 8 cores; core k handles batch b=k//2, i-half k%2 (128 i-rows x
256 j-cols of one batch's NxN grid).

Per-core plan (v5):
  - nf and the mean-selection matrix in fp8-e4m3 (error budget checked in
    sim; DMA bytes halve vs bf16). 5 nf chunks + weights balanced across
    3 trigger queues (sync HW, gpsimd SW, scalar HW) at ~256KB each.
  - hT via per-octet matmuls (fp8 stationary x fp8 moving) pipelined
    behind the DMA chunks, drained bf16 in j-128 halves; cT matmuls and
    ScalarE copies chunked so the first act op trails the last nf byte
    by ~1us.
  - aTb4 = (aT+b1) x4-replicated fp32 for the VE 16B-aligned scalar
    fast path.
  - Pairwise: act buffer [128, 1024] per pair-step = [ht0 i_a | ht0 i_b
    | ht1 i_a | ht1 i_b], filled by 4 ops split between VectorE
    (dual-op tensor_scalar ~203ns) and ScalarE (activation Relu+bias
    ~412ns) via greedy finish-time balancing.
  - Reduction: one matmul per (pair, ht): stationary [128,n_g] diag with
    w2_ht in column r', PSUM region [n_g, 512] per group; group sizes
    16/16/16/12/4 so drains overlap compute and the final
    drain+DMA tail is small.
  - b2 and final assembly applied on host.
"""

import os
import sys

import numpy as np

_B, _N, _T, _F, _H = 4, 256, 16, 128, 256
_NCORES = 8

_VE_NS = 202.7  # measured per [128,256] dual tensor_scalar
_SE_NS = 411.5  # measured per [128,256] activation relu+bias
_SE_DRAIN_NS = 720.0
_VE_DRAIN_NS = 658.0
_GP_NS = 1e9  # gpsimd act tiles disabled: in-context cost blew up

_GROUPS = [16, 16, 16, 12, 4]  # pairs per PSUM region

_CACHE = {}


def _ensure_paths():
    for p in (
        "/root/.axon_site",
        "/root/.axon_site/_ro/trn_rl_repo",
        "/root/.axon_site/_ro/pypackages",
        "/opt/trn_rl_repo",
    ):
        if os.path.isdir(p) and p not in sys.path:
            sys.path.append(p)


def build_nc():
    """Build the per-core Bass program (same program for all 8 cores)."""
    _ensure_paths()
    import concourse.mybir as mybir
    import concourse.tile as tile
    from concourse import bacc

    f32 = mybir.dt.float32
    bf16 = mybir.dt.bfloat16
    fp8 = mybir.dt.float8e4
    Alu = mybir.AluOpType
    Act = mybir.ActivationFunctionType

    nc = bacc.Bacc("TRN2", target_bir_lowering=False, debug=False)

    nf = nc.declare_dram_parameter("nf", [128, 32, 128], fp8, isOutput=False)
    smat = nc.declare_dram_parameter("smat", [128, 8], fp8, isOutput=False)
    # wpack[:, t, 0:128]=w1c_t^T, [:, t, 128:256]=w1a_t^T, [:, t, 256:512]=w2 diag
    wpack = nc.declare_dram_parameter("wpack", [128, 2, 512], bf16, isOutput=False)
    b1t = nc.declare_dram_parameter("b1t", [128, 2], f32, isOutput=False)
    outd = nc.declare_dram_parameter("out", [64, 512], f32, isOutput=True)

    with tile.TileContext(nc) as tc:
        with (
            tc.tile_pool(name="const", bufs=1) as constp,
            tc.tile_pool(name="data", bufs=1) as datap,
            tc.tile_pool(name="act", bufs=20) as actp,
            tc.tile_pool(name="dr", bufs=2) as drp,
            tc.tile_pool(name="ph", bufs=1, space="PSUM") as php,
            tc.tile_pool(name="pc", bufs=2, space="PSUM") as pcp,
            tc.tile_pool(name="pl", bufs=2, space="PSUM") as plp,
        ):
            smat_sb = constp.tile([128, 8], fp8, tag="smat")
            nc.sync.dma_start(out=smat_sb[:], in_=smat[:])

            nf_sb = constp.tile([128, 32, 128], fp8, tag="nf")
            wpack_sb = constp.tile([128, 2, 512], bf16, tag="wpack")
            b1t_sb = constp.tile([128, 2], f32, tag="b1t")
            # Queue plan (per-queue ~1.6us latency + ~85GB/s): b1t + the w1
            # half of wpack lead the scalar queue (needed first); nf split
            # ~byte-balanced across all three queues; w2 diag half arrives
            # before the first pairwise matmul.
            nc.scalar.dma_start(out=b1t_sb[:], in_=b1t[:])
            nc.scalar.dma_start(out=wpack_sb[:, :, 0:256], in_=wpack[:, :, 0:256])
            nc.sync.dma_start(out=nf_sb[:, 0:8, :], in_=nf[:, 0:8, :])
            nc.gpsimd.dma_start(out=nf_sb[:, 8:16, :], in_=nf[:, 8:16, :])
            nc.scalar.dma_start(out=nf_sb[:, 22:32, :], in_=nf[:, 22:32, :])
            nc.gpsimd.dma_start(out=nf_sb[:, 16:22, :], in_=nf[:, 16:22, :])
            nc.sync.dma_start(out=wpack_sb[:, :, 256:512], in_=wpack[:, :, 256:512])

            # split PSUM tiles so j-half consumers wait only their own half;
            # octet order follows expected DMA chunk arrival
            phA = php.tile([128, 128], f32, tag="phA")
            phB = php.tile([128, 128], f32, tag="phB")
            for o in range(32):
                ph = phA if o < 16 else phB
                nc.tensor.matmul(
                    ph[:, 8 * (o % 16) : 8 * (o % 16) + 8],
                    lhsT=nf_sb[:, o, :],
                    rhs=smat_sb[:],
                    start=True,
                    stop=True,
                )

            hTa = datap.tile([128, 128], bf16, tag="hTa")
            hTb = datap.tile([128, 128], bf16, tag="hTb")
            cT = [datap.tile([128, 256], bf16, tag=f"cT{t}", name=f"cT{t}") for t in range(2)]
            aTb4 = [datap.tile([128, 128, 4], f32, tag=f"aTb4{t}", name=f"aTb4{t}") for t in range(2)]
            # separate [128,128] PSUM tiles per (t, j-half): no false WAR deps
            pcs = [
                [pcp.tile([128, 128], f32, tag="pc", name=f"pc{t}h{h}") for h in range(2)]
                for t in range(2)
            ]

            # first j-half: hT drain, cT chunk mms, aT chain, cT copies
            nc.vector.tensor_copy(hTa[:], phA[:])
            for t in range(2):
                nc.tensor.matmul(
                    pcs[t][0][:], lhsT=wpack_sb[:, t, 0:128],
                    rhs=hTa[:], start=True, stop=True,
                )
            pa = [pcp.tile([128, 128], f32, tag="pa", name=f"pa{t}") for t in range(2)]
            for t in range(2):
                nc.tensor.matmul(
                    pa[t][:], lhsT=wpack_sb[:, t, 128:256],
                    rhs=hTa[:], start=True, stop=True,
                )
            # second j-half feeds through before the aTb4 build so the
            # critical path (cT[0] complete) clears ScalarE early
            nc.vector.tensor_copy(hTb[:], phB[:])
            for t in range(2):
                nc.tensor.matmul(
                    pcs[t][1][:], lhsT=wpack_sb[:, t, 0:128],
                    rhs=hTb[:], start=True, stop=True,
                )
            nc.scalar.copy(cT[0][:, 0:128], pcs[0][0][:])
            nc.scalar.copy(cT[0][:, 128:256], pcs[0][1][:])
            nc.vector.tensor_scalar(
                aTb4[0][:, :, :],
                pa[0][:].broadcast_to([128, 128, 4]),
                b1t_sb[:, 0:1],
                None,
                Alu.add,
            )
            nc.scalar.copy(cT[1][:, 0:128], pcs[1][0][:])
            nc.scalar.copy(cT[1][:, 128:256], pcs[1][1][:])
            nc.vector.tensor_scalar(
                aTb4[1][:, :, :],
                pa[1][:].broadcast_to([128, 128, 4]),
                b1t_sb[:, 1:2],
                None,
                Alu.add,
            )

            # Pairwise main loop over groups of pairs.  Each matmul's moving
            # operand is one [128,512] tile written by exactly one engine
            # (two ops) — no cross-engine intra-tile deps.  GpSimd takes an
            # occasional tile pair as a slow third engine.
            tV = 700.0  # aTb4[0] ahead of first act
            tS = 400.0  # cT copies
            tG = 0.0
            p0 = 0
            for g, ng in enumerate(_GROUPS):
                last_group = g == len(_GROUPS) - 1
                pl = plp.tile([ng, 512], f32, tag="pl", name=f"pl{g}")
                for rp in range(ng):
                    for t in range(2):
                        buf = actp.tile([128, 512], bf16, tag="act")
                        cV = tV + 2 * _VE_NS
                        cS = tS + 2 * _SE_NS
                        cG = tG + 2 * _GP_NS
                        best = min(cV, cS, cG)
                        if best == cV:
                            eng, tV = nc.vector, cV
                        elif best == cS:
                            eng, tS = nc.scalar, cS
                        else:
                            eng, tG = nc.gpsimd, cG
                        for s in range(2):
                            i = 2 * (p0 + rp) + s
                            a_col = aTb4[t][:, i, 0:1]
                            dst = buf[:, 256 * s : 256 * s + 256]
                            if eng is nc.scalar:
                                nc.scalar.activation(dst, cT[t][:], Act.Relu, bias=a_col)
                            else:
                                eng.tensor_scalar(
                                    dst, cT[t][:], a_col, 0.0, Alu.add, Alu.max
                                )
                        nc.tensor.matmul(
                            pl[:, :],
                            lhsT=wpack_sb[:, t, 256 + 16 * rp : 256 + 16 * rp + ng],
                            rhs=buf[:],
                            start=(rp == 0 and t == 0),
                            stop=(rp == ng - 1 and t == 1),
                        )
                osb = drp.tile([ng, 512], f32, tag="osb", name=f"osb{g}")
                if last_group or tV + _VE_DRAIN_NS <= tS + _SE_DRAIN_NS:
                    nc.vector.tensor_copy(osb[:], pl[:])
                    tV += _VE_DRAIN_NS
                else:
                    nc.scalar.copy(osb[:], pl[:])
                    tS += _SE_DRAIN_NS
                nc.sync.dma_start(out=outd[p0 : p0 + ng], in_=osb[:])
                p0 += ng

    nc.compile()
    return nc


def make_in_maps(nodefeat, W1, b1, W2, b2):
    """Host-side sharding/layout prep (layout + dtype only)."""
    import ml_dtypes

    bf16 = ml_dtypes.bfloat16
    fp8 = ml_dtypes.float8_e4m3fn
    nodefeat = np.asarray(nodefeat, dtype=np.float32)
    W1 = np.asarray(W1, dtype=np.float32)
    b1 = np.asarray(b1, dtype=np.float32)
    W2 = np.asarray(W2, dtype=np.float32)

    smat = (np.repeat(np.eye(8, dtype=np.float32), 16, axis=0) / 16.0).astype(fp8)

    W1a, W1c = W1[:, :_F], W1[:, _F:]
    w1at = np.stack([W1a[:128].T, W1a[128:].T], axis=1)  # [128 f, 2, 128 h]
    w1ct = np.stack([W1c[:128].T, W1c[128:].T], axis=1)
    b1t = np.ascontiguousarray(b1.reshape(2, 128).T)

    w2r = W2[0].reshape(2, 128)  # [ht, p]
    w2b = np.zeros((128, 2, 16, 16), dtype=np.float32)
    idx = np.arange(16)
    w2b[:, :, idx, idx] = w2r.T[:, :, None]

    wpack = np.concatenate(
        [w1ct, w1at, w2b.reshape(128, 2, 256)], axis=2
    ).astype(bf16)  # [128, 2, 512]

    # fp8 with error feedback along T: each slice is individually fp8-close
    # to its true value, and the T-sum the device computes stays accurate.
    nfq = np.empty_like(nodefeat)
    carry = np.zeros(nodefeat[:, :, 0, :].shape, dtype=np.float32)
    for t in range(_T):
        x = nodefeat[:, :, t, :] + carry
        qx = x.astype(fp8).astype(np.float32)
        carry = x - qx
        nfq[:, :, t, :] = qx

    in_maps = []
    for k in range(_NCORES):
        b, ih = divmod(k, 2)
        nf_b = nfq[b]  # [256, 16, 128]
        if ih:
            nf_b = np.concatenate([nf_b[128:], nf_b[:128]], axis=0)
        # [256,16,128] -> [32 oct, (j8,t16)=128, 128 f] -> [128, 32, 128]
        nf_dev = np.ascontiguousarray(
            nf_b.reshape(32, 128, 128).transpose(1, 0, 2).astype(fp8)
        )
        in_maps.append(
            {
                "nf": nf_dev,
                "smat": smat,
                "wpack": wpack,
                "b1t": b1t,
            }
        )
    return in_maps


def core_output_to_ij(arr, b2_val):
    """Device output [64, 512] -> core-local logits [128 i, 256 j]."""
    return arr.reshape(128, 256).astype(np.float32) + b2_val


def assemble_output(results, b2):
    b2_val = float(np.asarray(b2).reshape(-1)[0])
    out = np.empty((_B, _N, _N), dtype=np.float32)
    for k in range(_NCORES):
        b, ih = divmod(k, 2)
        r = core_output_to_ij(results[k]["out"], b2_val)  # [i, j] core-local j order
        if ih:
            r = np.concatenate([r[:, 128:], r[:, :128]], axis=1)
        out[b, ih * 128 : (ih + 1) * 128, :] = r
    return out


def _get_nc():
    if "nc" not in _CACHE:
        _CACHE["nc"] = build_nc()
    return _CACHE["nc"]


def kernel(nodefeat, W1, b1, W2, b2):
    _ensure_paths()
    from concourse.bass_utils import run_bass_kernel_spmd

    nc = _get_nc()
    in_maps = make_in_maps(nodefeat, W1, b1, W2, b2)
    res = run_bass_kernel_spmd(nc, in_maps, list(range(_NCORES)))
    return assemble_output(res.results, b2)
